# revision 2
# baseline (speedup 1.0000x reference)
"""Trainium2 Bass kernel for per-sample weight-demodulated 3x3 conv + leaky ReLU.

Problem (hardcoded shapes):
  input_vector: (8, 256, 128, 128) f32
  style_vector: (8, 256) f32
  weight:       (256, 256, 3, 3) f32
  out:          (8, 256, 128, 128) f32

Math (faithful to reference):
  ws[b,o,i,kh,kw] = weight[o,i,kh,kw] * style[b,i]
  demod[b,kw]     = rsqrt(sum_{o,i,kh} ws^2 + 1e-8)        # NOTE: sum excludes kw
  y[b] = leaky_relu(conv2d_same(x[b], ws[b]*demod), 0.2)

Sharding: data-parallel over batch, one sample per NeuronCore (8 cores).

Per-core kernel (optimized; TimelineSim ~263 us vs ~290 us baseline):
  - x and w shipped from host as fp16 (halves HBM traffic; conv accumulates in
    f32 PSUM — ~1e-3 rel err vs the 2e-2 gate). y returned fp16, host upcasts.
  - ALL DMAs ride the SP queue (HW showed ACT-queue DMA->PE semaphore edges
    losing races). Order: w first, stile, x pieces 0-2 (issued pre-prep so
    their transfers overlap weight prep), then pieces 3-7 + merged y drains at
    their conv sites.
  - PE warmup: throwaway ident transposes overlap the w-DMA wait, ramping the
    PE clock (HAM) and absorbing the identity-ready semaphore; a dummy sqrt at
    t=0 pulls the ACT function-set table load off the critical path.
  - Weight prep kw-major, big ops: per (kw, kb) the 3 kh x 2 mb [128,128]
    transposes land in one [128,768] f16 psum tile; one ACT activation(Copy,
    scale=style) scales into f32 wfin; ONE ACT Square+accum_out per (kw, kb)
    forms the demod sum of squares; ones-matmul broadcasts the cross-partition
    sum; eps/sqrt/recip; one DVE tensor_scalar rescale+fp16-cast per (kw, kb)
    into wfin16. Each kw's dps-matmul is emitted after the NEXT kw's
    transposes so it never blocks the PE FIFO.
  - x staged in 8 pieces of 18 padded rows (fp16, 130 wide, zero pad via
    memset), each piece in its OWN tile (no buffer reuse: rotating x buffers
    lost write-after-read races on HW).
  - Conv: per 4-row chunk (N=512), 36 fp16 matmuls accumulate into 2 f32 PSUM
    tiles in kw-major tap order, so chunk 0 streams right behind the per-kw
    weight rescales with no PE stall. Epilogue: leaky relu = max(x, 0.2x) on
    DVE, both mb written into
    one [128, 2, N] tile, drained by a single merged y DMA per chunk
    (GPSIMD cannot read PSUM, so DVE does both mb). The last piece ends with
    two 2-row sub-chunks so the drain tail is short.
"""

import numpy as np

B, CIN, COUT, K, H, W = 8, 256, 256, 3, 128, 128
P = 128
KB = CIN // P   # cin partition blocks   = 2
MB = COUT // P  # cout partition blocks  = 2
T = K * K       # taps = 9
WP = W + 2      # padded row width = 130
NP = 8          # x pieces
PROWS = H // NP           # output rows per piece = 16
PPAD = PROWS + 2          # padded rows held per piece = 18
CHUNK_ROWS = 4            # output rows per psum chunk
CHUNK_N = CHUNK_ROWS * W  # matmul free size = 512
CHUNKS_PER_PIECE = PROWS // CHUNK_ROWS  # = 4
N_WARMUP = 40

_CACHE = {}


def _build(stage="full"):
    import concourse.mybir as mybir
    import concourse.tile as tile
    from concourse import bacc
    from concourse.masks import make_identity

    f32 = mybir.dt.float32
    f16 = mybir.dt.float16

    nc = bacc.Bacc(None, target_bir_lowering=False)
    x_d = nc.dram_tensor("x", [CIN, H, W], f16, kind="ExternalInput")
    s_d = nc.dram_tensor("style", [1, CIN], f32, kind="ExternalInput")
    w_d = nc.dram_tensor("w", [COUT, CIN, K, K], f16, kind="ExternalInput")
    y_d = nc.dram_tensor("y", [COUT, H, W], f16, kind="ExternalOutput")

    y_flat = y_d[:].rearrange("o h w -> o (h w)")      # [256, 16384]
    y_pmf = y_d[:].rearrange("(m p) h w -> p m (h w)", p=P)  # [128, 2, 16384]
    w_flat = w_d[:].rearrange("o i kh kw -> o (i kh kw)")  # [256, 2304]

    with tile.TileContext(nc) as tc:
        with (
            tc.tile_pool(name="const", bufs=1) as const,
            tc.tile_pool(name="wtmp", bufs=1) as wtmp,
            tc.tile_pool(name="xbuf", bufs=1) as xbuf,
            tc.tile_pool(name="outp", bufs=3) as outp,
            tc.tile_pool(name="psum", bufs=2, space="PSUM") as psum,
            tc.tile_pool(name="psumw", bufs=2, space="PSUM") as psumw,
            tc.tile_pool(name="psumd", bufs=1, space="PSUM") as psumd,
        ):
            # ---------- constants ----------
            ident = const.tile([P, P], f16)
            make_identity(nc, ident)
            ones = const.tile([P, P], f32)
            nc.vector.memset(ones, 1.0)
            # dummy sqrt so the ACT function-set containing Sqrt loads now,
            # not mid-prep (the table swap drains the ACT pipeline)
            nc.scalar.sqrt(ones[0:1, 0:1], ones[0:1, 0:1])

            # ---------- weight load: very first DMA on the SP queue ----------
            wbuf = wtmp.tile([P, MB, CIN * T], f16)
            nc.sync.dma_start(
                out=wbuf[:],
                in_=w_flat.rearrange("(m p) f -> p m f", p=P),
            )

            # style per-partition: stile[p, kb] = style[kb*128 + p]
            stile = const.tile([P, KB], f32)
            for kb in range(KB):
                nc.sync.dma_start(
                    out=stile[:, kb : kb + 1],
                    in_=s_d[:].rearrange("one c -> c one")[kb * P : (kb + 1) * P, :],
                )

            # ---------- x pieces: alloc + pad + DMA ----------
            xqs = {}

            def stage_piece(p, dma_engine):
                # one tile per piece (no pool-buffer reuse: reused x buffers
                # showed WAR races on HW — sparse halo/edge corruption)
                xq = xbuf.tile([P, KB, PPAD, WP], f16, name=f"xq{p}")
                xqs[p] = xq
                img_lo = p * PROWS - 1
                img_hi = p * PROWS + PROWS  # inclusive
                lo_clip = max(img_lo, 0)
                hi_clip = min(img_hi, H - 1)
                l_lo = lo_clip - img_lo
                nrows = hi_clip - lo_clip + 1
                for kb in range(KB):
                    nc.vector.memset(xq[:, kb, :, 0], 0.0)
                    nc.vector.memset(xq[:, kb, :, WP - 1], 0.0)
                    if img_lo < 0:
                        nc.vector.memset(xq[:, kb, 0, :], 0.0)
                    if img_hi > H - 1:
                        nc.vector.memset(xq[:, kb, PPAD - 1, :], 0.0)
                    dma_engine.dma_start(
                        out=xq[:, kb, l_lo : l_lo + nrows, 1 : 1 + W],
                        in_=x_d[kb * P : (kb + 1) * P, lo_clip : hi_clip + 1, :],
                    )

            # pieces 0-2 issued early so their transfers ride behind w on SP
            for p in range(3):
                stage_piece(p, nc.sync)

            # ---------- PE warmup: ramp the clock while the w DMA flies ------
            for _ in range(N_WARMUP):
                gate = psumw.tile([P, K * MB * P], f16, name="pt")
                nc.tensor.transpose(gate[:, 0:P], ident, ident)

            # ---------- weight prep, kw-major ----------
            # wfin[i_part, kb, t, mb, o] f32 (style-scaled), wfin16 = *demod fp16
            wfin = const.tile([P, KB, T, MB, P], f32)
            wfin16 = const.tile([P, KB, T, MB, P], f16)
            wview = wbuf[:].rearrange("p m (i t) -> p m t i", t=T)  # strided view

            sp = wtmp.tile([P, KB, K], f32)
            spc = wtmp.tile([P, K], f32)
            junk = wtmp.tile([P, K * MB * P], f32)
            dps = psumd.tile([P, K], f32)
            demod = const.tile([P, K], f32)

            def emit_group(kw):
                # transpose the 6 (kh, mb) tiles of each kb into one psum tile,
                # then one style-scale op per kb, then the sum-of-squares
                for kb in range(KB):
                    pt = psumw.tile([P, K * MB * P], f16, name="pt")
                    ptv = pt[:].rearrange("p (kh mb o) -> p kh mb o", kh=K, mb=MB)
                    for kh in range(K):
                        t = kh * K + kw
                        for mb in range(MB):
                            nc.tensor.transpose(
                                ptv[:, kh, mb, :],
                                wview[:, mb, t, kb * P : (kb + 1) * P],
                                ident,
                            )
                    nc.scalar.activation(
                        out=wfin[:, kb, kw::K, :, :],
                        in_=ptv,
                        func=mybir.ActivationFunctionType.Copy,
                        scale=stile[:, kb : kb + 1],
                    )
                # demod[kw] numerator: sum of squares over (o, i, kh) in ONE
                # ACT op per kb: Square(wfin) with free-dim accumulator
                for kb in range(KB):
                    nc.scalar.activation(
                        out=junk[:].rearrange("p (a b c) -> p a b c", a=K, b=MB),
                        in_=wfin[:, kb, kw::K, :, :],
                        func=mybir.ActivationFunctionType.Square,
                        accum_out=sp[:, kb, kw : kw + 1],
                    )
                nc.vector.tensor_add(
                    out=spc[:, kw : kw + 1],
                    in0=sp[:, 0, kw : kw + 1],
                    in1=sp[:, 1, kw : kw + 1],
                )

            def emit_tail(kw):
                # cross-partition sum broadcast to all partitions, demod chain,
                # rescale + fp16 cast (one op per kb)
                nc.tensor.matmul(
                    dps[:, kw : kw + 1], ones, spc[:, kw : kw + 1],
                    start=True, stop=True,
                )
                nc.vector.tensor_scalar_add(
                    demod[:, kw : kw + 1], dps[:, kw : kw + 1], 1e-8
                )
                nc.scalar.sqrt(demod[:, kw : kw + 1], demod[:, kw : kw + 1])
                nc.vector.reciprocal(demod[:, kw : kw + 1], demod[:, kw : kw + 1])
                for kb in range(KB):
                    nc.vector.tensor_scalar_mul(
                        out=wfin16[:, kb, kw::K, :, :],
                        in0=wfin[:, kb, kw::K, :, :],
                        scalar1=demod[:, kw : kw + 1],
                    )

            # software-pipeline: each kw's dps-matmul is emitted after the NEXT
            # kw's transposes so it never blocks them in the PE FIFO
            emit_group(0)
            emit_group(1)
            emit_tail(0)
            emit_group(2)
            emit_tail(1)
            emit_tail(2)

            if stage == "wprep":
                ot = outp.tile([P, KB * T * MB * P], f16)
                nc.vector.tensor_copy(
                    out=ot, in_=wfin16[:].rearrange("p a b c d -> p (a b c d)")
                )
                nc.sync.dma_start(out=y_flat[0:P, 0 : KB * T * MB * P], in_=ot)
                ot2 = outp.tile([P, K], f16)
                nc.vector.tensor_copy(out=ot2, in_=demod)
                nc.sync.dma_start(out=y_flat[0:P, 16000 : 16000 + K], in_=ot2)

            if stage == "full":
                # ---------- conv over 8 pieces ----------
                # last chunk split in two (2-row) so the drain tail is shorter
                for p in range(NP):
                    if p >= 3:
                        stage_piece(p, nc.sync)
                    xq = xqs[p]
                    if p < NP - 1:
                        chunks = [(j * CHUNK_ROWS, CHUNK_ROWS)
                                  for j in range(CHUNKS_PER_PIECE)]
                    else:
                        chunks = [(j * CHUNK_ROWS, CHUNK_ROWS)
                                  for j in range(CHUNKS_PER_PIECE - 1)]
                        half = CHUNK_ROWS // 2
                        base = (CHUNKS_PER_PIECE - 1) * CHUNK_ROWS
                        chunks += [(base, half), (base + half, half)]
                    for lr0, nrows in chunks:
                        r0 = p * PROWS + lr0
                        n_free = nrows * W
                        pts = [
                            psum.tile([P, CHUNK_N], f32, name=f"pc{mb}")
                            for mb in range(MB)
                        ]
                        # kw-major tap order: chunk 0 streams behind the
                        # per-kw weight rescales with no PE stall
                        first = True
                        for kw in range(K):
                            for kb in range(KB):
                                for kh in range(K):
                                    t = kh * K + kw
                                    rhs = xq[
                                        :, kb, lr0 + kh : lr0 + kh + nrows,
                                        kw : kw + W,
                                    ]
                                    last = kw == K - 1 and kb == KB - 1 and kh == K - 1
                                    for mb in range(MB):
                                        nc.tensor.matmul(
                                            pts[mb][:, 0:n_free],
                                            wfin16[:, kb, t, mb, :],
                                            rhs,
                                            start=first,
                                            stop=last,
                                        )
                                    first = False
                        # leaky relu = max(x, 0.2x) on DVE (idle during conv),
                        # fp16 out; y DMA on SP. The short tail sub-chunks
                        # split mb across DVE/GPSIMD so the final drain chain
                        # is half as long.
                        ot = outp.tile([P, MB, CHUNK_N], f16, name="ot")
                        for mb in range(MB):
                            tmp = outp.tile([P, CHUNK_N], f32, name=f"lt{mb}")
                            nc.vector.tensor_scalar_mul(
                                tmp[:, 0:n_free], pts[mb][:, 0:n_free], 0.2
                            )
                            nc.vector.tensor_tensor(
                                out=ot[:, mb, 0:n_free], in0=pts[mb][:, 0:n_free],
                                in1=tmp[:, 0:n_free],
                                op=mybir.AluOpType.max,
                            )
                        nc.sync.dma_start(
                            out=y_pmf[:, :, r0 * W : r0 * W + n_free],
                            in_=ot[:, :, 0:n_free],
                        )
    nc.compile()
    return nc


def _get_nc():
    if "nc" not in _CACHE:
        _CACHE["nc"] = _build()
    return _CACHE["nc"]


def prep_in_maps(input_vector, style_vector, weight):
    """Host-side staging: fp16 casts, per-core input dicts."""
    x16 = np.ascontiguousarray(input_vector, dtype=np.float16)
    w16 = np.ascontiguousarray(weight, dtype=np.float16)
    s32 = np.ascontiguousarray(style_vector, dtype=np.float32)
    return [
        {"x": x16[b], "style": s32[b : b + 1], "w": w16}
        for b in range(B)
    ]


def kernel(input_vector, style_vector, weight):
    from concourse.bass_utils import run_bass_kernel_spmd

    nc = _get_nc()
    in_maps = prep_in_maps(input_vector, style_vector, weight)
    res = run_bass_kernel_spmd(nc, in_maps, core_ids=list(range(B)))
    out = np.stack([res.results[b]["y"] for b in range(B)], axis=0)
    return out.astype(np.float32)


# revision 3
# speedup vs baseline: 1.0281x; 1.0281x over previous
"""Trainium2 Bass kernel for per-sample weight-demodulated 3x3 conv + leaky ReLU.

Problem (hardcoded shapes):
  input_vector: (8, 256, 128, 128) f32
  style_vector: (8, 256) f32
  weight:       (256, 256, 3, 3) f32
  out:          (8, 256, 128, 128) f32

Math (faithful to reference):
  ws[b,o,i,kh,kw] = weight[o,i,kh,kw] * style[b,i]
  demod[b,kw]     = rsqrt(sum_{o,i,kh} ws^2 + 1e-8)        # NOTE: sum excludes kw
  y[b] = leaky_relu(conv2d_same(x[b], ws[b]*demod), 0.2)

Sharding: data-parallel over batch, one sample per NeuronCore (8 cores).

Per-core kernel (optimized; TimelineSim ~263 us vs ~290 us baseline):
  - x and w shipped from host as fp16 (halves HBM traffic; conv accumulates in
    f32 PSUM — ~1e-3 rel err vs the 2e-2 gate). y returned fp16, host upcasts.
  - ALL DMAs ride the SP queue (HW showed ACT-queue DMA->PE semaphore edges
    losing races). Order: w first, stile, x pieces 0-2 (issued pre-prep so
    their transfers overlap weight prep), then pieces 3-7 + merged y drains at
    their conv sites.
  - PE warmup: throwaway ident transposes overlap the w-DMA wait, ramping the
    PE clock (HAM) and absorbing the identity-ready semaphore; a dummy sqrt at
    t=0 pulls the ACT function-set table load off the critical path.
  - Weight prep kw-major, big ops: per (kw, kb) the 3 kh x 2 mb [128,128]
    transposes land in one [128,768] f16 psum tile; one ACT activation(Copy,
    scale=style) scales into f32 wfin; ONE ACT Square+accum_out per (kw, kb)
    forms the demod sum of squares; ones-matmul broadcasts the cross-partition
    sum; eps/sqrt/recip; one DVE tensor_scalar rescale+fp16-cast per (kw, kb)
    into wfin16. Each kw's dps-matmul is emitted after the NEXT kw's
    transposes so it never blocks the PE FIFO.
  - x staged in 8 pieces of 18 padded rows (fp16, 130 wide, zero pad via
    memset), each piece in its OWN tile (no buffer reuse: rotating x buffers
    lost write-after-read races on HW).
  - Conv: per 4-row chunk (N=512), 36 fp16 matmuls accumulate into 2 f32 PSUM
    tiles in kw-major tap order, so chunk 0 streams right behind the per-kw
    weight rescales with no PE stall. Epilogue: leaky relu = max(x, 0.2x) on
    DVE, both mb written into
    one [128, 2, N] tile, drained by a single merged y DMA per chunk
    (GPSIMD cannot read PSUM, so DVE does both mb). The last piece ends with
    two 2-row sub-chunks so the drain tail is short.
"""

import numpy as np

B, CIN, COUT, K, H, W = 8, 256, 256, 3, 128, 128
P = 128
KB = CIN // P   # cin partition blocks   = 2
MB = COUT // P  # cout partition blocks  = 2
T = K * K       # taps = 9
WP = W + 2      # padded row width = 130
NP = 8          # x pieces
PROWS = H // NP           # output rows per piece = 16
PPAD = PROWS + 2          # padded rows held per piece = 18
CHUNK_ROWS = 4            # output rows per psum chunk
CHUNK_N = CHUNK_ROWS * W  # matmul free size = 512
CHUNKS_PER_PIECE = PROWS // CHUNK_ROWS  # = 4
N_WARMUP = 40

_CACHE = {}


def _build(stage="full"):
    import concourse.mybir as mybir
    import concourse.tile as tile
    from concourse import bacc
    from concourse.masks import make_identity

    f32 = mybir.dt.float32
    f16 = mybir.dt.float16

    nc = bacc.Bacc(None, target_bir_lowering=False)
    x_d = nc.dram_tensor("x", [CIN, H, W], f16, kind="ExternalInput")
    s_d = nc.dram_tensor("style", [1, CIN], f32, kind="ExternalInput")
    w_d = nc.dram_tensor("w", [COUT, CIN, K, K], f16, kind="ExternalInput")
    y_d = nc.dram_tensor("y", [COUT, H, W], f16, kind="ExternalOutput")

    y_flat = y_d[:].rearrange("o h w -> o (h w)")      # [256, 16384]
    y_pmf = y_d[:].rearrange("(m p) h w -> p m (h w)", p=P)  # [128, 2, 16384]
    w_flat = w_d[:].rearrange("o i kh kw -> o (i kh kw)")  # [256, 2304]

    with tile.TileContext(nc) as tc:
        with (
            tc.tile_pool(name="const", bufs=1) as const,
            tc.tile_pool(name="wtmp", bufs=1) as wtmp,
            tc.tile_pool(name="xbuf", bufs=1) as xbuf,
            tc.tile_pool(name="outp", bufs=3) as outp,
            tc.tile_pool(name="psum", bufs=2, space="PSUM") as psum,
            tc.tile_pool(name="psumw", bufs=2, space="PSUM") as psumw,
            tc.tile_pool(name="psumd", bufs=1, space="PSUM") as psumd,
        ):
            # ---------- constants ----------
            ident = const.tile([P, P], f16)
            make_identity(nc, ident)
            ones = const.tile([P, P], f32)
            nc.vector.memset(ones, 1.0)
            # dummy sqrt so the ACT function-set containing Sqrt loads now,
            # not mid-prep (the table swap drains the ACT pipeline)
            nc.scalar.sqrt(ones[0:1, 0:1], ones[0:1, 0:1])

            # ---------- weight load: very first DMA on the SP queue ----------
            wbuf = wtmp.tile([P, MB, CIN * T], f16)
            nc.sync.dma_start(
                out=wbuf[:],
                in_=w_flat.rearrange("(m p) f -> p m f", p=P),
            )

            # style per-partition: stile[p, kb] = style[kb*128 + p]
            stile = const.tile([P, KB], f32)
            for kb in range(KB):
                nc.sync.dma_start(
                    out=stile[:, kb : kb + 1],
                    in_=s_d[:].rearrange("one c -> c one")[kb * P : (kb + 1) * P, :],
                )

            # ---------- x pieces: alloc + pad + DMA ----------
            xqs = {}

            def stage_piece(p, dma_engine):
                # one tile per piece (no pool-buffer reuse: reused x buffers
                # showed WAR races on HW — sparse halo/edge corruption)
                xq = xbuf.tile([P, KB, PPAD, WP], f16, name=f"xq{p}")
                xqs[p] = xq
                img_lo = p * PROWS - 1
                img_hi = p * PROWS + PROWS  # inclusive
                lo_clip = max(img_lo, 0)
                hi_clip = min(img_hi, H - 1)
                l_lo = lo_clip - img_lo
                nrows = hi_clip - lo_clip + 1
                for kb in range(KB):
                    nc.vector.memset(xq[:, kb, :, 0], 0.0)
                    nc.vector.memset(xq[:, kb, :, WP - 1], 0.0)
                    if img_lo < 0:
                        nc.vector.memset(xq[:, kb, 0, :], 0.0)
                    if img_hi > H - 1:
                        nc.vector.memset(xq[:, kb, PPAD - 1, :], 0.0)
                    dma_engine.dma_start(
                        out=xq[:, kb, l_lo : l_lo + nrows, 1 : 1 + W],
                        in_=x_d[kb * P : (kb + 1) * P, lo_clip : hi_clip + 1, :],
                    )

            # pieces 0-2 issued early so their transfers ride behind w on SP
            for p in range(3):
                stage_piece(p, nc.sync)

            # ---------- PE warmup: ramp the clock while the w DMA flies ------
            for _ in range(N_WARMUP):
                gate = psumw.tile([P, K * MB * P], f16, name="pt")
                nc.tensor.transpose(gate[:, 0:P], ident, ident)

            # ---------- weight prep, kw-major ----------
            # wfin[i_part, kb, t, mb, o] f32 (style-scaled), wfin16 = *demod fp16
            wfin = const.tile([P, KB, T, MB, P], f32)
            wfin16 = const.tile([P, KB, T, MB, P], f16)
            wview = wbuf[:].rearrange("p m (i t) -> p m t i", t=T)  # strided view

            sp = wtmp.tile([P, KB, K], f32)
            spc = wtmp.tile([P, K], f32)
            junk = wtmp.tile([P, K * MB * P], f32)
            dps = psumd.tile([P, K], f32)
            demod = const.tile([P, K], f32)

            def emit_group(kw):
                # transpose the 6 (kh, mb) tiles of each kb into one psum tile,
                # then one style-scale op per kb, then the sum-of-squares
                for kb in range(KB):
                    pt = psumw.tile([P, K * MB * P], f16, name="pt")
                    ptv = pt[:].rearrange("p (kh mb o) -> p kh mb o", kh=K, mb=MB)
                    for kh in range(K):
                        t = kh * K + kw
                        for mb in range(MB):
                            nc.tensor.transpose(
                                ptv[:, kh, mb, :],
                                wview[:, mb, t, kb * P : (kb + 1) * P],
                                ident,
                            )
                    nc.scalar.activation(
                        out=wfin[:, kb, kw::K, :, :],
                        in_=ptv,
                        func=mybir.ActivationFunctionType.Copy,
                        scale=stile[:, kb : kb + 1],
                    )
                # demod[kw] numerator: sum of squares over (o, i, kh) in ONE
                # ACT op per kb: Square(wfin) with free-dim accumulator
                for kb in range(KB):
                    nc.scalar.activation(
                        out=junk[:].rearrange("p (a b c) -> p a b c", a=K, b=MB),
                        in_=wfin[:, kb, kw::K, :, :],
                        func=mybir.ActivationFunctionType.Square,
                        accum_out=sp[:, kb, kw : kw + 1],
                    )
                nc.vector.tensor_add(
                    out=spc[:, kw : kw + 1],
                    in0=sp[:, 0, kw : kw + 1],
                    in1=sp[:, 1, kw : kw + 1],
                )

            def emit_tail(kw):
                # cross-partition sum broadcast to all partitions, demod chain,
                # rescale + fp16 cast (one op per kb)
                nc.tensor.matmul(
                    dps[:, kw : kw + 1], ones, spc[:, kw : kw + 1],
                    start=True, stop=True,
                )
                nc.vector.tensor_scalar_add(
                    demod[:, kw : kw + 1], dps[:, kw : kw + 1], 1e-8
                )
                nc.scalar.sqrt(demod[:, kw : kw + 1], demod[:, kw : kw + 1])
                nc.vector.reciprocal(demod[:, kw : kw + 1], demod[:, kw : kw + 1])
                for kb in range(KB):
                    nc.vector.tensor_scalar_mul(
                        out=wfin16[:, kb, kw::K, :, :],
                        in0=wfin[:, kb, kw::K, :, :],
                        scalar1=demod[:, kw : kw + 1],
                    )

            # software-pipeline: each kw's dps-matmul is emitted after the NEXT
            # kw's transposes so it never blocks them in the PE FIFO
            emit_group(0)
            emit_group(1)
            emit_tail(0)
            emit_group(2)
            emit_tail(1)
            emit_tail(2)

            if stage == "wprep":
                ot = outp.tile([P, KB * T * MB * P], f16)
                nc.vector.tensor_copy(
                    out=ot, in_=wfin16[:].rearrange("p a b c d -> p (a b c d)")
                )
                nc.sync.dma_start(out=y_flat[0:P, 0 : KB * T * MB * P], in_=ot)
                ot2 = outp.tile([P, K], f16)
                nc.vector.tensor_copy(out=ot2, in_=demod)
                nc.sync.dma_start(out=y_flat[0:P, 16000 : 16000 + K], in_=ot2)

            if stage == "full":
                # ---------- conv over 8 pieces ----------
                # last chunk split in two (2-row) so the drain tail is shorter
                for p in range(NP):
                    if p >= 3:
                        stage_piece(p, nc.sync)
                    xq = xqs[p]
                    if p < NP - 1:
                        chunks = [(j * CHUNK_ROWS, CHUNK_ROWS)
                                  for j in range(CHUNKS_PER_PIECE)]
                    else:
                        chunks = [(j * CHUNK_ROWS, CHUNK_ROWS)
                                  for j in range(CHUNKS_PER_PIECE - 1)]
                        half = CHUNK_ROWS // 2
                        base = (CHUNKS_PER_PIECE - 1) * CHUNK_ROWS
                        chunks += [(base, half), (base + half, half)]
                    for lr0, nrows in chunks:
                        r0 = p * PROWS + lr0
                        n_free = nrows * W
                        pts = [
                            psum.tile([P, CHUNK_N], f32, name=f"pc{mb}")
                            for mb in range(MB)
                        ]
                        # kw-major tap order: chunk 0 streams behind the
                        # per-kw weight rescales with no PE stall
                        first = True
                        for kw in range(K):
                            for kb in range(KB):
                                for kh in range(K):
                                    t = kh * K + kw
                                    rhs = xq[
                                        :, kb, lr0 + kh : lr0 + kh + nrows,
                                        kw : kw + W,
                                    ]
                                    last = kw == K - 1 and kb == KB - 1 and kh == K - 1
                                    for mb in range(MB):
                                        nc.tensor.matmul(
                                            pts[mb][:, 0:n_free],
                                            wfin16[:, kb, t, mb, :],
                                            rhs,
                                            start=first,
                                            stop=last,
                                        )
                                    first = False
                        # leaky relu = max(x, 0.2x) on DVE (idle during conv),
                        # fp16 out; y DMA on SP. The short tail sub-chunks
                        # split mb across DVE/GPSIMD so the final drain chain
                        # is half as long.
                        ot = outp.tile([P, MB, CHUNK_N], f16, name="ot")
                        for mb in range(MB):
                            tmp = outp.tile([P, CHUNK_N], f32, name=f"lt{mb}")
                            nc.vector.tensor_scalar_mul(
                                tmp[:, 0:n_free], pts[mb][:, 0:n_free], 0.2
                            )
                            nc.vector.tensor_tensor(
                                out=ot[:, mb, 0:n_free], in0=pts[mb][:, 0:n_free],
                                in1=tmp[:, 0:n_free],
                                op=mybir.AluOpType.max,
                            )
                        nc.sync.dma_start(
                            out=y_pmf[:, :, r0 * W : r0 * W + n_free],
                            in_=ot[:, :, 0:n_free],
                        )
    nc.compile()
    return nc


def _get_nc():
    if "nc" not in _CACHE:
        _CACHE["nc"] = _build()
    return _CACHE["nc"]


def prep_in_maps(input_vector, style_vector, weight):
    """Host-side staging: fp16 casts, per-core input dicts."""
    x16 = np.ascontiguousarray(input_vector, dtype=np.float16)
    w16 = np.ascontiguousarray(weight, dtype=np.float16)
    s32 = np.ascontiguousarray(style_vector, dtype=np.float32)
    return [
        {"x": x16[b], "style": s32[b : b + 1], "w": w16}
        for b in range(B)
    ]


def _get_runner():
    """Build (once) a reusable jitted shard_map runner over the 8 cores, so
    repeated kernel() calls skip re-tracing/lowering the bass module."""
    if "runner" in _CACHE:
        return _CACHE["runner"]

    import jax
    import concourse.bass2jax as b2j
    import concourse.mybir as mybir
    from jax.experimental.shard_map import shard_map
    from jax.sharding import Mesh, PartitionSpec

    nc = _get_nc()
    b2j.install_neuronx_cc_hook()

    partition_name = nc.partition_id_tensor.name if nc.partition_id_tensor else None
    in_names, out_names, out_avals, zero_outs = [], [], [], []
    for alloc in nc.m.functions[0].allocations:
        if not isinstance(alloc, mybir.MemoryLocationSet):
            continue
        name = alloc.memorylocations[0].name
        if alloc.kind == "ExternalInput":
            if name != partition_name:
                in_names.append(name)
        elif alloc.kind == "ExternalOutput":
            out_names.append(name)
            shape = tuple(alloc.tensor_shape)
            dtype = mybir.dt.np(alloc.dtype)
            out_avals.append(jax.core.ShapedArray(shape, dtype))
            zero_outs.append(np.zeros(shape, dtype))
    n_params = len(in_names)
    n_outs = len(out_avals)
    all_in_names = list(in_names) + list(out_names)
    if partition_name is not None:
        all_in_names.append(partition_name)

    def _body(*args):
        operands = list(args)
        if partition_name is not None:
            operands.append(b2j.partition_id_tensor())
        outs = b2j._bass_exec_p.bind(
            *operands,
            out_avals=tuple(out_avals),
            in_names=tuple(all_in_names),
            out_names=tuple(out_names),
            lowering_input_output_aliases=(),
            sim_require_finite=True,
            sim_require_nnan=True,
            nc=nc,
        )
        return tuple(outs)

    devices = jax.devices()[:B]
    mesh = Mesh(np.asarray(devices), ("core",))
    in_specs = (PartitionSpec("core"),) * (n_params + n_outs)
    out_specs = (PartitionSpec("core"),) * len(out_names)
    sharded = jax.jit(
        shard_map(_body, mesh=mesh, in_specs=in_specs, out_specs=out_specs,
                  check_rep=False),
        donate_argnums=tuple(range(n_params, n_params + n_outs)),
        keep_unused=True,
    )
    _CACHE["runner"] = (sharded, in_names, out_names, out_avals, zero_outs)
    return _CACHE["runner"]


def kernel(input_vector, style_vector, weight):
    in_maps = prep_in_maps(input_vector, style_vector, weight)
    try:
        sharded, in_names, out_names, out_avals, zero_outs = _get_runner()
        concat_in = [
            np.concatenate([in_maps[c][nm] for c in range(B)], axis=0)
            for nm in in_names
        ]
        zeros = [
            np.zeros((B * z.shape[0], *z.shape[1:]), z.dtype) for z in zero_outs
        ]
        out_arrs = sharded(*concat_in, *zeros)
        yi = out_names.index("y")
        out = np.asarray(out_arrs[yi]).reshape(B, *out_avals[yi].shape)
    except Exception:
        # fallback: the one-shot path (slower per call, same result)
        from concourse.bass_utils import run_bass_kernel_spmd

        _CACHE.pop("runner", None)
        res = run_bass_kernel_spmd(_get_nc(), in_maps, core_ids=list(range(B)))
        out = np.stack([res.results[b]["y"] for b in range(B)], axis=0)
    return out.astype(np.float32)
